# revision 10
# baseline (speedup 1.0000x reference)
"""Trainium2 Bass kernel for nn_DiagSSMBlock.

Math: s = x @ B  (T=4096, H=2048); h_t = a * h_{t-1} + s_t per channel
(equivalent to the reference depthwise causal conv with kernel a^t, since
|a| <= sqrt(2/H) ~= 0.031 the kernel decays below fp32 denormals within
~16 taps).  Output: (1, T, H).

Sharding: data-parallel over T across 8 cores; each core computes 512
timesteps (plus W=4 warm-up rows to rebuild the scan carry; a^5 ~ 3e-8
makes the truncation error ~1e-7, far under the 2e-2 gate).  Every core
streams the full B.

Measured-design notes (HW traces):
  - x/B in bf16: PE streams 1 column/cycle for fp32r and bf16 alike, so
    GEMM time is unchanged, but DMA bytes halve and LDWEIGHTS uses the
    fast-weight-load path (~97ns, fully hidden under ~111ns matmuls).
  - The early phase is supply-bound: the two HWDGE rings share the
    ~358 GB/s HBM-per-core cap, and each dma_start costs ~0.65us of
    descriptor-gen.  Phase 1 therefore runs THREE m-tiles over k-HALF
    blocks (PSUM accumulation groups stay open), so only xp0-3 +
    half-B-tiles gate the start; the supply plan interleaves both rings
    in exact PE-consumption order.
  - Up-front filler matmuls (memset tile -> psA0, reset by m0k0's
    start=True) warm the PE HAM clock gate during the ~7us framework
    preamble + DMA ramp.
  - Output is written bf16 and widened to fp32 on the host; rounding
    error ~0.4% of |h|, well under the 2e-2 gate.
  - DVE ops have ~390ns fixed overhead -> one scan per PSUM half, one
    output DMA per m-tile (the last tile is split for tail latency).

Per-core device pipeline:
  - x chunk is pre-transposed on the host (sharding layout prep) into
    xT[p, k, t] = x[t, 128k + p], so the GEMM contraction dim lands on
    SBUF partitions with no on-device transpose.
  - GEMM: for each of 16 output-channel tiles m, accumulate 16 k-tile
    matmuls into PSUM (bf16 operands, fp32 accumulate, moving free dim
    258 >= 256 -> full PE rate).
  - Scan: tensor_tensor_scan (DVE) state = a*state + s straight out of
    PSUM into SBUF, chained across the two 258-wide chunks.
  - Output stays channel-major (h^T) on device; the host unshard
    restores (T, H) layout while gathering the 8 T-chunks.
"""

from contextlib import ExitStack

import numpy as np

T_FULL, H = 4096, 2048
N_CORES = 8
T_CHUNK = T_FULL // N_CORES  # 512
W = 4  # scan warm-up rows
T_SPAN = T_CHUNK + W  # 516
HALF = T_SPAN // 2  # 258 (>= 256 keeps matmul at full rate)
KT = H // 128  # 16 contraction tiles
MT = H // 128  # 16 output-channel tiles
NP = 8  # xT arrives as 8 two-slab pieces
PH1 = 3  # phase-1 m-tiles (2 PSUM banks each, k-half blocks)
N_WARM = 16  # up-front HAM warm-up filler matmuls (N=258 each)

_CACHE = {}


def _build():
    import concourse.mybir as mybir
    import concourse.tile as tile
    from concourse import bacc

    f32 = mybir.dt.float32
    bf16 = mybir.dt.bfloat16

    nc = bacc.Bacc("TRN2", target_bir_lowering=False, debug=False, num_devices=N_CORES)
    xT = nc.dram_tensor("xT", [128, KT, T_SPAN], bf16, kind="ExternalInput").ap()
    Bm = nc.dram_tensor("Bm", [MT, 128, KT, 128], bf16, kind="ExternalInput").ap()
    a = nc.dram_tensor("a", [128, MT], f32, kind="ExternalInput").ap()
    out = nc.dram_tensor("out", [MT, 128, T_CHUNK], bf16, kind="ExternalOutput").ap()

    with tile.TileContext(nc) as tc, ExitStack() as ctx:
        const = ctx.enter_context(tc.tile_pool(name="const", bufs=1))
        xt_pool = ctx.enter_context(tc.tile_pool(name="xt", bufs=NP))
        b_pool = ctx.enter_context(tc.tile_pool(name="bm", bufs=MT))
        ht_pool = ctx.enter_context(tc.tile_pool(name="ht", bufs=6))
        ps_gemm = ctx.enter_context(tc.tile_pool(name="psg", bufs=8, space="PSUM"))

        rings = [nc.sync, nc.scalar]

        bms = {}
        xps = [None] * NP

        def load_xp(p, ring):
            t = xt_pool.tile([128, 2 * T_SPAN], bf16, tag="xt", name=f"xp{p}")
            ring.dma_start(
                out=t[:].rearrange("p (k t) -> p k t", k=2),
                in_=xT[:, 2 * p : 2 * p + 2, :],
            )
            xps[p] = t

        def load_bm(m, ring, lo=0, hi=KT):
            if m not in bms:
                bms[m] = b_pool.tile([128, KT * 128], bf16, tag="bm", name=f"bm{m}")
            ring.dma_start(
                out=bms[m][:, lo * 128 : hi * 128].rearrange(
                    "p (k c) -> p k c", k=hi - lo
                ),
                in_=Bm[m, :, lo:hi, :],
            )

        # Supply plan in PE-consumption order across both rings (A=sync,
        # B=scalar).  Phase-1 needs xp0-3 + the k0-7 halves of bm0-2
        # first; the k8-15 halves and bm3+ stream behind.
        a_sb = const.tile([128, MT], f32)
        load_bm(0, nc.sync, 0, 8)       # A: bm0a
        load_xp(0, nc.scalar)           # B: xp0
        load_xp(1, nc.sync)             # A: xp1
        load_xp(2, nc.gpsimd)           # SWDGE third queue: rescues the
        nc.gpsimd.dma_start(out=a_sb, in_=a)  # early-supply shortfall
        load_xp(3, nc.scalar)           # B: xp3
        load_bm(1, nc.sync, 0, 8)       # A: bm1a
        load_bm(2, nc.scalar, 0, 8)     # B: bm2a
        load_xp(4, nc.sync)             # A: xp4
        load_xp(5, nc.scalar)           # B: xp5
        load_xp(6, nc.sync)             # A: xp6
        load_bm(0, nc.scalar, 8, 16)    # B: bm0b
        load_xp(7, nc.sync)             # A: xp7
        load_bm(1, nc.scalar, 8, 16)    # B: bm1b
        load_bm(2, nc.sync, 8, 16)      # A: bm2b
        load_bm(3, nc.scalar)           # B
        load_bm(4, nc.sync)             # A
        load_bm(5, nc.scalar)           # B
        load_bm(6, nc.sync)             # A

        def xt_slice(k, lo, hi):
            return xps[k // 2][:, (k % 2) * T_SPAN + lo : (k % 2) * T_SPAN + hi]

        def emit_mm(ps, m, k, lo, hi):
            nc.tensor.matmul(
                ps[:],
                bms[m][:, k * 128 : (k + 1) * 128],
                xt_slice(k, lo, hi),
                start=(k == 0),
                stop=(k == KT - 1),
            )

        def emit_scan_out(m, psA, psB):
            ht = ht_pool.tile([128, T_SPAN], bf16, tag="ht", name=f"ht{m}")
            a_bc = a_sb[:, m : m + 1].broadcast_to([128, HALF])
            nc.vector.tensor_tensor_scan(
                ht[:, 0:HALF], a_bc, psA[:], 0.0,
                mybir.AluOpType.mult, mybir.AluOpType.add,
            )
            if m < MT - 1:
                nc.vector.tensor_tensor_scan(
                    ht[:, HALF:T_SPAN], a_bc, psB[:], ht[:, HALF - 1 : HALF],
                    mybir.AluOpType.mult, mybir.AluOpType.add,
                )
                rings[m % 2].dma_start(
                    out=out[m, :, :], in_=ht[:, W:T_SPAN]
                )
            else:
                # last m-tile: store the first half as soon as its scan is
                # done and split the trailing scan+store so the final
                # dependency chain after the last matmul is short
                rings[m % 2].dma_start(
                    out=out[m, :, 0 : HALF - W], in_=ht[:, W:HALF]
                )
                q3 = HALF + HALF // 2
                a_bc_h = a_sb[:, m : m + 1].broadcast_to([128, HALF // 2])
                nc.vector.tensor_tensor_scan(
                    ht[:, HALF:q3], a_bc_h, psB[:, 0 : HALF // 2],
                    ht[:, HALF - 1 : HALF],
                    mybir.AluOpType.mult, mybir.AluOpType.add,
                )
                rings[m % 2].dma_start(
                    out=out[m, :, HALF - W : q3 - W], in_=ht[:, HALF:q3]
                )
                nc.vector.tensor_tensor_scan(
                    ht[:, q3:T_SPAN], a_bc_h, psB[:, HALF // 2 : HALF],
                    ht[:, q3 - 1 : q3],
                    mybir.AluOpType.mult, mybir.AluOpType.add,
                )
                rings[(m + 1) % 2].dma_start(
                    out=out[m, :, q3 - W : T_CHUNK], in_=ht[:, q3:T_SPAN]
                )

        # PSUM tiles for phase-1 (A/B halves per m; accumulation groups
        # stay open across the k-half blocks).
        ph1 = {}
        for m in range(PH1):
            ph1[m] = (
                ps_gemm.tile([128, HALF], f32, tag="ps", name=f"psA{m}"),
                ps_gemm.tile([128, HALF], f32, tag="ps", name=f"psB{m}"),
            )

        # Up-front HAM warm-up: filler matmuls on a memset bf16 tile with
        # no DMA dependency, targeting psA0 — m0k0's start=True resets it.
        warm = const.tile([128, HALF], bf16)
        nc.gpsimd.memset(warm, 0.0)
        for _ in range(N_WARM):
            nc.tensor.matmul(
                ph1[0][0][:], warm[:, 0:128], warm[:], start=True, stop=True
            )
        # Spare bank for mid-phase bridge fillers (phase-1 banks all hold
        # open accumulations, so bridges need their own target).
        ps_bridge = ps_gemm.tile([128, HALF], f32, tag="ps", name="ps_bridge")

        def bridge(n):
            # Small fillers that keep the PE busy (and the HAM clock gate
            # warm) across a supply-paced stall; drain at ~56ns each if
            # the data is already resident.
            for _ in range(n):
                nc.tensor.matmul(
                    ps_bridge[:, 0:128], warm[:, 0:128], warm[:, 0:128],
                    start=True, stop=True,
                )

        # Phase 1: m0-m2 in k-half blocks following the x pieces.
        for m in range(PH1):
            for k in range(8):
                emit_mm(ph1[m][0], m, k, 0, HALF)
                emit_mm(ph1[m][1], m, k, HALF, T_SPAN)
            if m < 2:
                bridge(10 if m == 0 else 6)
        for m in range(PH1):
            for k in range(8, KT):
                emit_mm(ph1[m][0], m, k, 0, HALF)
                emit_mm(ph1[m][1], m, k, HALF, T_SPAN)
        for m in range(PH1):
            emit_scan_out(m, *ph1[m])

        # Phase 2: remaining m-tiles run dense, k-inner; B tiles stream
        # three m ahead, alternating rings.
        for m in range(PH1, MT):
            if m + 4 < MT:
                load_bm(m + 4, rings[(m + 4) % 2])
            psA = ps_gemm.tile([128, HALF], f32, tag="ps", name=f"psA{m}")
            psB = ps_gemm.tile([128, HALF], f32, tag="ps", name=f"psB{m}")
            for k in range(KT):
                emit_mm(psA, m, k, 0, HALF)
            for k in range(KT):
                emit_mm(psB, m, k, HALF, T_SPAN)
            emit_scan_out(m, psA, psB)

    nc.compile()
    return nc


def _get_nc():
    if "nc" not in _CACHE:
        _CACHE["nc"] = _build()
    return _CACHE["nc"]


def _shard_inputs(x, a, B):
    import ml_dtypes

    bf16 = ml_dtypes.bfloat16
    x = np.ascontiguousarray(x, dtype=np.float32)
    a = np.ascontiguousarray(a, dtype=np.float32)
    B = np.ascontiguousarray(B, dtype=np.float32)
    B_lin = np.ascontiguousarray(
        B.reshape(KT, 128, MT, 128).transpose(2, 1, 0, 3).astype(bf16)
    )  # [m, p, k, c] = B[128k+p, 128m+c]
    a_lin = np.ascontiguousarray(a.reshape(MT, 128).T)  # [p, m] = a[128m+p]
    xp = np.concatenate([np.zeros((W, H), np.float32), x], axis=0).astype(bf16)
    in_maps = []
    for c in range(N_CORES):
        chunk = xp[c * T_CHUNK : c * T_CHUNK + T_SPAN]  # (T_SPAN, H)
        xT_lin = np.ascontiguousarray(
            chunk.T.reshape(KT, 128, T_SPAN).transpose(1, 0, 2)
        )  # [p, k, t] = x[t, 128k+p]
        in_maps.append({"xT": xT_lin, "Bm": B_lin, "a": a_lin})
    return in_maps


def _gather_output(results):
    out = np.empty((T_FULL, H), np.float32)
    for c in range(N_CORES):
        o = np.asarray(results[c]["out"], dtype=np.float32)  # (MT, 128, T_CHUNK)
        out[c * T_CHUNK : (c + 1) * T_CHUNK] = o.reshape(H, T_CHUNK).T
    return out[None]


def _run(inputs, trace=False):
    from concourse import bass_utils

    nc = _get_nc()
    in_maps = _shard_inputs(inputs["x"], inputs["a"], inputs["B"])
    res = bass_utils.run_bass_kernel_spmd(
        nc, in_maps, core_ids=list(range(N_CORES)), trace=trace
    )
    return _gather_output(res.results), res


def kernel(x, a, B):
    out, _ = _run({"x": x, "a": a, "B": B})
    return out


# revision 12
# speedup vs baseline: 1.0014x; 1.0014x over previous
"""Trainium2 Bass kernel for nn_DiagSSMBlock.

Math: s = x @ B  (T=4096, H=2048); h_t = a * h_{t-1} + s_t per channel
(equivalent to the reference depthwise causal conv with kernel a^t, since
|a| <= sqrt(2/H) ~= 0.031 the kernel decays below fp32 denormals within
~16 taps).  Output: (1, T, H).

Sharding: data-parallel over T across 8 cores; each core computes 512
timesteps (plus W=4 warm-up rows to rebuild the scan carry; a^5 ~ 3e-8
makes the truncation error ~1e-7, far under the 2e-2 gate).  Every core
streams the full B.

Measured-design notes (HW traces):
  - x/B in bf16: PE streams 1 column/cycle for fp32r and bf16 alike, so
    GEMM time is unchanged, but DMA bytes halve and LDWEIGHTS uses the
    fast-weight-load path (~97ns, fully hidden under ~111ns matmuls).
  - The early phase is supply-bound: the two HWDGE rings share the
    ~358 GB/s HBM-per-core cap, and each dma_start costs ~0.65us of
    descriptor-gen.  Phase 1 therefore runs THREE m-tiles over k-HALF
    blocks (PSUM accumulation groups stay open), so only xp0-3 +
    half-B-tiles gate the start; the supply plan interleaves both rings
    in exact PE-consumption order.
  - Up-front filler matmuls (memset tile -> psA0, reset by m0k0's
    start=True) warm the PE HAM clock gate during the ~7us framework
    preamble + DMA ramp.
  - Output is written bf16 and widened to fp32 on the host; rounding
    error ~0.4% of |h|, well under the 2e-2 gate.
  - DVE ops have ~390ns fixed overhead -> one scan per PSUM half, one
    output DMA per m-tile (the last tile is split for tail latency).

Per-core device pipeline:
  - x chunk is pre-transposed on the host (sharding layout prep) into
    xT[p, k, t] = x[t, 128k + p], so the GEMM contraction dim lands on
    SBUF partitions with no on-device transpose.
  - GEMM: for each of 16 output-channel tiles m, accumulate 16 k-tile
    matmuls into PSUM (bf16 operands, fp32 accumulate, moving free dim
    258 >= 256 -> full PE rate).
  - Scan: tensor_tensor_scan (DVE) state = a*state + s straight out of
    PSUM into SBUF, chained across the two 258-wide chunks.
  - Output stays channel-major (h^T) on device; the host unshard
    restores (T, H) layout while gathering the 8 T-chunks.
"""

from contextlib import ExitStack

import numpy as np

T_FULL, H = 4096, 2048
N_CORES = 8
T_CHUNK = T_FULL // N_CORES  # 512
W = 4  # scan warm-up rows
T_SPAN = T_CHUNK + W  # 516
HALF = T_SPAN // 2  # 258 (>= 256 keeps matmul at full rate)
KT = H // 128  # 16 contraction tiles
MT = H // 128  # 16 output-channel tiles
NP = 8  # xT arrives as 8 two-slab pieces
PH1 = 3  # phase-1 m-tiles (2 PSUM banks each, k-half blocks)
N_WARM = 16  # up-front HAM warm-up filler matmuls (N=258 each)

_CACHE = {}


def _build():
    import concourse.mybir as mybir
    import concourse.tile as tile
    from concourse import bacc

    f32 = mybir.dt.float32
    bf16 = mybir.dt.bfloat16

    nc = bacc.Bacc("TRN2", target_bir_lowering=False, debug=False, num_devices=N_CORES)
    xT = nc.dram_tensor("xT", [128, KT, T_SPAN], bf16, kind="ExternalInput").ap()
    Bm = nc.dram_tensor("Bm", [MT, 128, KT, 128], bf16, kind="ExternalInput").ap()
    a = nc.dram_tensor("a", [128, MT], f32, kind="ExternalInput").ap()
    out = nc.dram_tensor("out", [MT, 128, T_CHUNK], bf16, kind="ExternalOutput").ap()

    with tile.TileContext(nc) as tc, ExitStack() as ctx:
        const = ctx.enter_context(tc.tile_pool(name="const", bufs=1))
        xt_pool = ctx.enter_context(tc.tile_pool(name="xt", bufs=NP))
        b_pool = ctx.enter_context(tc.tile_pool(name="bm", bufs=MT))
        ht_pool = ctx.enter_context(tc.tile_pool(name="ht", bufs=6))
        ps_gemm = ctx.enter_context(tc.tile_pool(name="psg", bufs=8, space="PSUM"))

        rings = [nc.sync, nc.scalar]

        bms = {}
        xps = [None] * NP

        def load_xp(p, ring):
            t = xt_pool.tile([128, 2 * T_SPAN], bf16, tag="xt", name=f"xp{p}")
            ring.dma_start(
                out=t[:].rearrange("p (k t) -> p k t", k=2),
                in_=xT[:, 2 * p : 2 * p + 2, :],
            )
            xps[p] = t

        def load_bm(m, ring, lo=0, hi=KT):
            if m not in bms:
                bms[m] = b_pool.tile([128, KT * 128], bf16, tag="bm", name=f"bm{m}")
            ring.dma_start(
                out=bms[m][:, lo * 128 : hi * 128].rearrange(
                    "p (k c) -> p k c", k=hi - lo
                ),
                in_=Bm[m, :, lo:hi, :],
            )

        # Supply plan in PE-consumption order across both rings (A=sync,
        # B=scalar).  Phase-1 needs xp0-3 + the k0-7 halves of bm0-2
        # first; the k8-15 halves and bm3+ stream behind.
        # The sync queue starts moving data ~0.9us before scalar, so the
        # earliest-needed items go on sync and strictly alternate after.
        a_sb = const.tile([128, MT], f32)
        load_bm(0, nc.sync, 0, 8)       # A: bm0a
        nc.scalar.dma_start(out=a_sb, in_=a)  # B: a (tiny)
        load_xp(0, nc.sync)             # A: xp0
        load_xp(1, nc.scalar)           # B: xp1
        load_xp(2, nc.sync)             # A: xp2
        load_xp(3, nc.scalar)           # B: xp3
        load_bm(1, nc.sync, 0, 8)       # A: bm1a
        load_bm(2, nc.scalar, 0, 8)     # B: bm2a
        load_xp(4, nc.sync)             # A: xp4
        load_xp(5, nc.scalar)           # B: xp5
        load_xp(6, nc.sync)             # A: xp6
        load_xp(7, nc.scalar)           # B: xp7
        load_bm(0, nc.sync, 8, 16)      # A: bm0b
        load_bm(1, nc.scalar, 8, 16)    # B: bm1b
        load_bm(2, nc.sync, 8, 16)      # A: bm2b
        load_bm(3, nc.scalar)           # B
        load_bm(4, nc.sync)             # A
        load_bm(5, nc.scalar)           # B
        load_bm(6, nc.sync)             # A

        def xt_slice(k, lo, hi):
            return xps[k // 2][:, (k % 2) * T_SPAN + lo : (k % 2) * T_SPAN + hi]

        def emit_mm(ps, m, k, lo, hi):
            nc.tensor.matmul(
                ps[:],
                bms[m][:, k * 128 : (k + 1) * 128],
                xt_slice(k, lo, hi),
                start=(k == 0),
                stop=(k == KT - 1),
            )

        def emit_scan_out(m, psA, psB):
            ht = ht_pool.tile([128, T_SPAN], bf16, tag="ht", name=f"ht{m}")
            a_bc = a_sb[:, m : m + 1].broadcast_to([128, HALF])
            nc.vector.tensor_tensor_scan(
                ht[:, 0:HALF], a_bc, psA[:], 0.0,
                mybir.AluOpType.mult, mybir.AluOpType.add,
            )
            if m < MT - 1:
                nc.vector.tensor_tensor_scan(
                    ht[:, HALF:T_SPAN], a_bc, psB[:], ht[:, HALF - 1 : HALF],
                    mybir.AluOpType.mult, mybir.AluOpType.add,
                )
                rings[m % 2].dma_start(
                    out=out[m, :, :], in_=ht[:, W:T_SPAN]
                )
            else:
                # last m-tile: store the first half as soon as its scan is
                # done and split the trailing scan+store so the final
                # dependency chain after the last matmul is short
                rings[m % 2].dma_start(
                    out=out[m, :, 0 : HALF - W], in_=ht[:, W:HALF]
                )
                q3 = HALF + HALF // 2
                a_bc_h = a_sb[:, m : m + 1].broadcast_to([128, HALF // 2])
                nc.vector.tensor_tensor_scan(
                    ht[:, HALF:q3], a_bc_h, psB[:, 0 : HALF // 2],
                    ht[:, HALF - 1 : HALF],
                    mybir.AluOpType.mult, mybir.AluOpType.add,
                )
                rings[m % 2].dma_start(
                    out=out[m, :, HALF - W : q3 - W], in_=ht[:, HALF:q3]
                )
                nc.vector.tensor_tensor_scan(
                    ht[:, q3:T_SPAN], a_bc_h, psB[:, HALF // 2 : HALF],
                    ht[:, q3 - 1 : q3],
                    mybir.AluOpType.mult, mybir.AluOpType.add,
                )
                rings[(m + 1) % 2].dma_start(
                    out=out[m, :, q3 - W : T_CHUNK], in_=ht[:, q3:T_SPAN]
                )

        # PSUM tiles for phase-1 (A/B halves per m; accumulation groups
        # stay open across the k-half blocks).
        ph1 = {}
        for m in range(PH1):
            ph1[m] = (
                ps_gemm.tile([128, HALF], f32, tag="ps", name=f"psA{m}"),
                ps_gemm.tile([128, HALF], f32, tag="ps", name=f"psB{m}"),
            )

        # Up-front HAM warm-up: filler matmuls on a memset bf16 tile with
        # no DMA dependency, targeting psA0 — m0k0's start=True resets it.
        warm = const.tile([128, HALF], bf16)
        nc.gpsimd.memset(warm, 0.0)
        for _ in range(N_WARM):
            nc.tensor.matmul(
                ph1[0][0][:], warm[:, 0:128], warm[:], start=True, stop=True
            )
        # Spare bank for mid-phase bridge fillers (phase-1 banks all hold
        # open accumulations, so bridges need their own target).
        ps_bridge = ps_gemm.tile([128, HALF], f32, tag="ps", name="ps_bridge")

        def bridge(n):
            # Small fillers that keep the PE busy (and the HAM clock gate
            # warm) across a supply-paced stall; drain at ~56ns each if
            # the data is already resident.
            for _ in range(n):
                nc.tensor.matmul(
                    ps_bridge[:, 0:128], warm[:, 0:128], warm[:, 0:128],
                    start=True, stop=True,
                )

        # Phase 1: m0-m2 in k-half blocks following the x pieces.
        for m in range(PH1):
            for k in range(8):
                emit_mm(ph1[m][0], m, k, 0, HALF)
                emit_mm(ph1[m][1], m, k, HALF, T_SPAN)
            bridge(8 if m == 0 else 0)
        bridge(8)  # before the k8-15 half: bm0b is still in flight
        for m in range(PH1):
            for k in range(8, KT):
                emit_mm(ph1[m][0], m, k, 0, HALF)
                emit_mm(ph1[m][1], m, k, HALF, T_SPAN)
        for m in range(PH1):
            emit_scan_out(m, *ph1[m])

        # Phase 2: remaining m-tiles run dense, k-inner; B tiles stream
        # three m ahead, alternating rings.
        for m in range(PH1, MT):
            if m + 4 < MT:
                load_bm(m + 4, rings[(m + 4) % 2])
            psA = ps_gemm.tile([128, HALF], f32, tag="ps", name=f"psA{m}")
            psB = ps_gemm.tile([128, HALF], f32, tag="ps", name=f"psB{m}")
            for k in range(KT):
                emit_mm(psA, m, k, 0, HALF)
            for k in range(KT):
                emit_mm(psB, m, k, HALF, T_SPAN)
            emit_scan_out(m, psA, psB)

    nc.compile()
    return nc


def _get_nc():
    if "nc" not in _CACHE:
        _CACHE["nc"] = _build()
    return _CACHE["nc"]


def _shard_inputs(x, a, B):
    import ml_dtypes

    bf16 = ml_dtypes.bfloat16
    x = np.ascontiguousarray(x, dtype=np.float32)
    a = np.ascontiguousarray(a, dtype=np.float32)
    B = np.ascontiguousarray(B, dtype=np.float32)
    B_lin = np.ascontiguousarray(
        B.reshape(KT, 128, MT, 128).transpose(2, 1, 0, 3).astype(bf16)
    )  # [m, p, k, c] = B[128k+p, 128m+c]
    a_lin = np.ascontiguousarray(a.reshape(MT, 128).T)  # [p, m] = a[128m+p]
    xp = np.concatenate([np.zeros((W, H), np.float32), x], axis=0).astype(bf16)
    in_maps = []
    for c in range(N_CORES):
        chunk = xp[c * T_CHUNK : c * T_CHUNK + T_SPAN]  # (T_SPAN, H)
        xT_lin = np.ascontiguousarray(
            chunk.T.reshape(KT, 128, T_SPAN).transpose(1, 0, 2)
        )  # [p, k, t] = x[t, 128k+p]
        in_maps.append({"xT": xT_lin, "Bm": B_lin, "a": a_lin})
    return in_maps


def _gather_output(results):
    out = np.empty((T_FULL, H), np.float32)
    for c in range(N_CORES):
        o = np.asarray(results[c]["out"], dtype=np.float32)  # (MT, 128, T_CHUNK)
        out[c * T_CHUNK : (c + 1) * T_CHUNK] = o.reshape(H, T_CHUNK).T
    return out[None]


def _run(inputs, trace=False):
    from concourse import bass_utils

    nc = _get_nc()
    in_maps = _shard_inputs(inputs["x"], inputs["a"], inputs["B"])
    res = bass_utils.run_bass_kernel_spmd(
        nc, in_maps, core_ids=list(range(N_CORES)), trace=trace
    )
    return _gather_output(res.results), res


def kernel(x, a, B):
    out, _ = _run({"x": x, "a": a, "B": B})
    return out


# revision 14
# speedup vs baseline: 1.0055x; 1.0041x over previous
"""Trainium2 Bass kernel for nn_DiagSSMBlock.

Math: s = x @ B  (T=4096, H=2048); h_t = a * h_{t-1} + s_t per channel
(equivalent to the reference depthwise causal conv with kernel a^t, since
|a| <= sqrt(2/H) ~= 0.031 the kernel decays below fp32 denormals within
~16 taps).  Output: (1, T, H).

Sharding: data-parallel over T across 8 cores; each core computes 512
timesteps (plus W=4 warm-up rows to rebuild the scan carry; a^5 ~ 3e-8
makes the truncation error ~1e-7, far under the 2e-2 gate).  Every core
streams the full B.

Measured-design notes (HW traces):
  - x/B in bf16: PE streams 1 column/cycle for fp32r and bf16 alike, so
    GEMM time is unchanged, but DMA bytes halve and LDWEIGHTS uses the
    fast-weight-load path (~97ns, fully hidden under ~111ns matmuls).
  - The early phase is supply-bound: the two HWDGE rings share the
    ~358 GB/s HBM-per-core cap, and each dma_start costs ~0.65us of
    descriptor-gen.  Phase 1 therefore runs THREE m-tiles over k-HALF
    blocks (PSUM accumulation groups stay open), so only xp0-3 +
    half-B-tiles gate the start; the supply plan interleaves both rings
    in exact PE-consumption order.
  - Up-front filler matmuls (memset tile -> psA0, reset by m0k0's
    start=True) warm the PE HAM clock gate during the ~7us framework
    preamble + DMA ramp.
  - Output is written bf16 and widened to fp32 on the host; rounding
    error ~0.4% of |h|, well under the 2e-2 gate.
  - DVE ops have ~390ns fixed overhead -> one scan per PSUM half, one
    output DMA per m-tile (the last tile is split for tail latency).

Per-core device pipeline:
  - x chunk is pre-transposed on the host (sharding layout prep) into
    xT[p, k, t] = x[t, 128k + p], so the GEMM contraction dim lands on
    SBUF partitions with no on-device transpose.
  - GEMM: for each of 16 output-channel tiles m, accumulate 16 k-tile
    matmuls into PSUM (bf16 operands, fp32 accumulate, moving free dim
    258 >= 256 -> full PE rate).
  - Scan: tensor_tensor_scan (DVE) state = a*state + s straight out of
    PSUM into SBUF, chained across the two 258-wide chunks.
  - Output stays channel-major (h^T) on device; the host unshard
    restores (T, H) layout while gathering the 8 T-chunks.
"""

from contextlib import ExitStack

import numpy as np

T_FULL, H = 4096, 2048
N_CORES = 8
T_CHUNK = T_FULL // N_CORES  # 512
W = 4  # scan warm-up rows
T_SPAN = T_CHUNK + W  # 516
HALF = T_SPAN // 2  # 258 (>= 256 keeps matmul at full rate)
KT = H // 128  # 16 contraction tiles
MT = H // 128  # 16 output-channel tiles
NP = 8  # xT arrives as 8 two-slab pieces
PH1 = 3  # phase-1 m-tiles (2 PSUM banks each, k-half blocks)
N_WARM = 16  # up-front HAM warm-up filler matmuls (N=258 each)

_CACHE = {}


def _build():
    import concourse.mybir as mybir
    import concourse.tile as tile
    from concourse import bacc

    f32 = mybir.dt.float32
    bf16 = mybir.dt.bfloat16

    nc = bacc.Bacc("TRN2", target_bir_lowering=False, debug=False, num_devices=N_CORES)
    xT = nc.dram_tensor("xT", [128, KT, T_SPAN], bf16, kind="ExternalInput").ap()
    Bm = nc.dram_tensor("Bm", [MT, 128, KT, 128], bf16, kind="ExternalInput").ap()
    a = nc.dram_tensor("a", [128, MT], f32, kind="ExternalInput").ap()
    out = nc.dram_tensor("out", [MT, 128, T_CHUNK], bf16, kind="ExternalOutput").ap()

    with tile.TileContext(nc) as tc, ExitStack() as ctx:
        const = ctx.enter_context(tc.tile_pool(name="const", bufs=1))
        xt_pool = ctx.enter_context(tc.tile_pool(name="xt", bufs=NP))
        b_pool = ctx.enter_context(tc.tile_pool(name="bm", bufs=MT))
        ht_pool = ctx.enter_context(tc.tile_pool(name="ht", bufs=6))
        ps_gemm = ctx.enter_context(tc.tile_pool(name="psg", bufs=8, space="PSUM"))

        rings = [nc.sync, nc.scalar]

        bms = {}
        xps = [None] * NP

        def load_xp(p, ring):
            t = xt_pool.tile([128, 2 * T_SPAN], bf16, tag="xt", name=f"xp{p}")
            ring.dma_start(
                out=t[:].rearrange("p (k t) -> p k t", k=2),
                in_=xT[:, 2 * p : 2 * p + 2, :],
            )
            xps[p] = t

        def load_bm(m, ring, lo=0, hi=KT):
            if m not in bms:
                bms[m] = b_pool.tile([128, KT * 128], bf16, tag="bm", name=f"bm{m}")
            ring.dma_start(
                out=bms[m][:, lo * 128 : hi * 128].rearrange(
                    "p (k c) -> p k c", k=hi - lo
                ),
                in_=Bm[m, :, lo:hi, :],
            )

        # Supply plan in PE-consumption order across both rings (A=sync,
        # B=scalar).  Phase-1 needs xp0-3 + the k0-7 halves of bm0-2
        # first; the k8-15 halves and bm3+ stream behind.
        # The sync queue starts moving data ~1-2us before scalar (and the
        # scalar queue's ramp is unreliable run-to-run), so the entire
        # phase-1a critical chain rides sync in consumption order; scalar
        # carries only items with later deadlines.
        a_sb = const.tile([128, MT], f32)
        load_bm(0, nc.sync, 0, 8)       # A: bm0a
        nc.scalar.dma_start(out=a_sb, in_=a)  # B: a (tiny)
        load_xp(0, nc.sync)             # A: xp0
        load_bm(2, nc.scalar, 0, 8)     # B: bm2a  (needed ~16us)
        load_xp(1, nc.sync)             # A: xp1
        load_xp(5, nc.scalar)           # B: xp5   (needed ~18us)
        load_xp(2, nc.sync)             # A: xp2
        load_xp(7, nc.scalar)           # B: xp7   (needed ~19us)
        load_xp(3, nc.sync)             # A: xp3
        load_bm(1, nc.scalar, 8, 16)    # B: bm1b  (needed ~19us)
        load_bm(1, nc.sync, 0, 8)       # A: bm1a
        load_bm(3, nc.scalar)           # B: bm3   (needed ~21us)
        load_xp(4, nc.sync)             # A: xp4
        load_xp(6, nc.sync)             # A: xp6
        load_bm(0, nc.sync, 8, 16)      # A: bm0b
        load_bm(2, nc.sync, 8, 16)      # A: bm2b
        load_bm(4, nc.scalar)           # B
        load_bm(5, nc.sync)             # A
        load_bm(6, nc.scalar)           # B

        def xt_slice(k, lo, hi):
            return xps[k // 2][:, (k % 2) * T_SPAN + lo : (k % 2) * T_SPAN + hi]

        def emit_mm(ps, m, k, lo, hi):
            nc.tensor.matmul(
                ps[:],
                bms[m][:, k * 128 : (k + 1) * 128],
                xt_slice(k, lo, hi),
                start=(k == 0),
                stop=(k == KT - 1),
            )

        def emit_scan_out(m, psA, psB):
            ht = ht_pool.tile([128, T_SPAN], bf16, tag="ht", name=f"ht{m}")
            a_bc = a_sb[:, m : m + 1].broadcast_to([128, HALF])
            nc.vector.tensor_tensor_scan(
                ht[:, 0:HALF], a_bc, psA[:], 0.0,
                mybir.AluOpType.mult, mybir.AluOpType.add,
            )
            if m < MT - 1:
                nc.vector.tensor_tensor_scan(
                    ht[:, HALF:T_SPAN], a_bc, psB[:], ht[:, HALF - 1 : HALF],
                    mybir.AluOpType.mult, mybir.AluOpType.add,
                )
                rings[m % 2].dma_start(
                    out=out[m, :, :], in_=ht[:, W:T_SPAN]
                )
            else:
                # last m-tile: store the first half as soon as its scan is
                # done and split the trailing scan+store so the final
                # dependency chain after the last matmul is short
                rings[m % 2].dma_start(
                    out=out[m, :, 0 : HALF - W], in_=ht[:, W:HALF]
                )
                q3 = HALF + HALF // 2
                a_bc_h = a_sb[:, m : m + 1].broadcast_to([128, HALF // 2])
                nc.vector.tensor_tensor_scan(
                    ht[:, HALF:q3], a_bc_h, psB[:, 0 : HALF // 2],
                    ht[:, HALF - 1 : HALF],
                    mybir.AluOpType.mult, mybir.AluOpType.add,
                )
                rings[m % 2].dma_start(
                    out=out[m, :, HALF - W : q3 - W], in_=ht[:, HALF:q3]
                )
                nc.vector.tensor_tensor_scan(
                    ht[:, q3:T_SPAN], a_bc_h, psB[:, HALF // 2 : HALF],
                    ht[:, q3 - 1 : q3],
                    mybir.AluOpType.mult, mybir.AluOpType.add,
                )
                rings[(m + 1) % 2].dma_start(
                    out=out[m, :, q3 - W : T_CHUNK], in_=ht[:, q3:T_SPAN]
                )

        # PSUM tiles for phase-1 (A/B halves per m; accumulation groups
        # stay open across the k-half blocks).
        ph1 = {}
        for m in range(PH1):
            ph1[m] = (
                ps_gemm.tile([128, HALF], f32, tag="ps", name=f"psA{m}"),
                ps_gemm.tile([128, HALF], f32, tag="ps", name=f"psB{m}"),
            )

        # Up-front HAM warm-up: filler matmuls on a memset bf16 tile with
        # no DMA dependency, targeting psA0 — m0k0's start=True resets it.
        warm = const.tile([128, HALF], bf16)
        nc.gpsimd.memset(warm, 0.0)
        for _ in range(N_WARM):
            nc.tensor.matmul(
                ph1[0][0][:], warm[:, 0:128], warm[:], start=True, stop=True
            )
        # Spare bank for mid-phase bridge fillers (phase-1 banks all hold
        # open accumulations, so bridges need their own target).
        ps_bridge = ps_gemm.tile([128, HALF], f32, tag="ps", name="ps_bridge")

        def bridge(n):
            # Small fillers that keep the PE busy (and the HAM clock gate
            # warm) across a supply-paced stall; drain at ~56ns each if
            # the data is already resident.
            for _ in range(n):
                nc.tensor.matmul(
                    ps_bridge[:, 0:128], warm[:, 0:128], warm[:, 0:128],
                    start=True, stop=True,
                )

        # Phase 1: m0-m2 in k-half blocks following the x pieces.
        for m in range(PH1):
            for k in range(8):
                emit_mm(ph1[m][0], m, k, 0, HALF)
                emit_mm(ph1[m][1], m, k, HALF, T_SPAN)
            bridge(6)
        for m in range(PH1):
            for k in range(8, KT):
                emit_mm(ph1[m][0], m, k, 0, HALF)
                emit_mm(ph1[m][1], m, k, HALF, T_SPAN)
        for m in range(PH1):
            emit_scan_out(m, *ph1[m])

        # Phase 2: remaining m-tiles run dense, k-inner; B tiles stream
        # three m ahead, alternating rings.
        for m in range(PH1, MT):
            if m + 4 < MT:
                load_bm(m + 4, rings[(m + 4) % 2])
            psA = ps_gemm.tile([128, HALF], f32, tag="ps", name=f"psA{m}")
            psB = ps_gemm.tile([128, HALF], f32, tag="ps", name=f"psB{m}")
            for k in range(KT):
                emit_mm(psA, m, k, 0, HALF)
            for k in range(KT):
                emit_mm(psB, m, k, HALF, T_SPAN)
            emit_scan_out(m, psA, psB)

    nc.compile()
    return nc


def _get_nc():
    if "nc" not in _CACHE:
        _CACHE["nc"] = _build()
    return _CACHE["nc"]


def _shard_inputs(x, a, B):
    import ml_dtypes

    bf16 = ml_dtypes.bfloat16
    x = np.ascontiguousarray(x, dtype=np.float32)
    a = np.ascontiguousarray(a, dtype=np.float32)
    B = np.ascontiguousarray(B, dtype=np.float32)
    B_lin = np.ascontiguousarray(
        B.reshape(KT, 128, MT, 128).transpose(2, 1, 0, 3).astype(bf16)
    )  # [m, p, k, c] = B[128k+p, 128m+c]
    a_lin = np.ascontiguousarray(a.reshape(MT, 128).T)  # [p, m] = a[128m+p]
    xp = np.concatenate([np.zeros((W, H), np.float32), x], axis=0).astype(bf16)
    in_maps = []
    for c in range(N_CORES):
        chunk = xp[c * T_CHUNK : c * T_CHUNK + T_SPAN]  # (T_SPAN, H)
        xT_lin = np.ascontiguousarray(
            chunk.T.reshape(KT, 128, T_SPAN).transpose(1, 0, 2)
        )  # [p, k, t] = x[t, 128k+p]
        in_maps.append({"xT": xT_lin, "Bm": B_lin, "a": a_lin})
    return in_maps


def _gather_output(results):
    out = np.empty((T_FULL, H), np.float32)
    for c in range(N_CORES):
        o = np.asarray(results[c]["out"], dtype=np.float32)  # (MT, 128, T_CHUNK)
        out[c * T_CHUNK : (c + 1) * T_CHUNK] = o.reshape(H, T_CHUNK).T
    return out[None]


def _run(inputs, trace=False):
    from concourse import bass_utils

    nc = _get_nc()
    in_maps = _shard_inputs(inputs["x"], inputs["a"], inputs["B"])
    res = bass_utils.run_bass_kernel_spmd(
        nc, in_maps, core_ids=list(range(N_CORES)), trace=trace
    )
    return _gather_output(res.results), res


def kernel(x, a, B):
    out, _ = _run({"x": x, "a": a, "B": B})
    return out


# revision 17
# speedup vs baseline: 1.0349x; 1.0292x over previous
"""Trainium2 Bass kernel for nn_DiagSSMBlock.

Math: s = x @ B  (T=4096, H=2048); h_t = a * h_{t-1} + s_t per channel
(equivalent to the reference depthwise causal conv with kernel a^t, since
|a| <= sqrt(2/H) ~= 0.031 the kernel decays below fp32 denormals within
~16 taps).  Output: (1, T, H).

Sharding: data-parallel over T across 8 cores; each core computes 512
timesteps (plus W=4 warm-up rows to rebuild the scan carry; a^5 ~ 3e-8
makes the truncation error ~1e-7, far under the 2e-2 gate).  Every core
streams the full B.

Measured-design notes (HW traces):
  - x/B in bf16: PE streams 1 column/cycle for fp32r and bf16 alike, so
    GEMM time is unchanged, but DMA bytes halve and LDWEIGHTS uses the
    fast-weight-load path (~97ns, fully hidden under ~111ns matmuls).
  - The early phase is supply-bound: the two HWDGE rings share the
    ~358 GB/s HBM-per-core cap, and each dma_start costs ~0.65us of
    descriptor-gen.  Phase 1 therefore runs THREE m-tiles over k-HALF
    blocks (PSUM accumulation groups stay open), so only xp0-3 +
    half-B-tiles gate the start; the supply plan interleaves both rings
    in exact PE-consumption order.
  - Up-front filler matmuls (memset tile -> psA0, reset by m0k0's
    start=True) warm the PE HAM clock gate during the ~7us framework
    preamble + DMA ramp.
  - Output is written bf16 and widened to fp32 on the host; rounding
    error ~0.4% of |h|, well under the 2e-2 gate.
  - DVE ops have ~390ns fixed overhead -> one scan per PSUM half, one
    output DMA per m-tile (the last tile is split for tail latency).

Per-core device pipeline:
  - x chunk is pre-transposed on the host (sharding layout prep) into
    xT[p, k, t] = x[t, 128k + p], so the GEMM contraction dim lands on
    SBUF partitions with no on-device transpose.
  - GEMM: for each of 16 output-channel tiles m, accumulate 16 k-tile
    matmuls into PSUM (bf16 operands, fp32 accumulate, moving free dim
    258 >= 256 -> full PE rate).
  - Scan: tensor_tensor_scan (DVE) state = a*state + s straight out of
    PSUM into SBUF, chained across the two 258-wide chunks.
  - Output stays channel-major (h^T) on device; the host unshard
    restores (T, H) layout while gathering the 8 T-chunks.
"""

from contextlib import ExitStack

import numpy as np

T_FULL, H = 4096, 2048
N_CORES = 8
T_CHUNK = T_FULL // N_CORES  # 512
W = 4  # scan warm-up rows
T_SPAN = T_CHUNK + W  # 516
HALF = T_SPAN // 2  # 258 (>= 256 keeps matmul at full rate)
KT = H // 128  # 16 contraction tiles
MT = H // 128  # 16 output-channel tiles
NP = 8  # xT arrives as 8 two-slab pieces
PH1 = 3  # phase-1 m-tiles (2 PSUM banks each, k-half blocks)
N_WARM = 14  # up-front HAM warm-up filler matmuls (N=258 each)

_CACHE = {}


def _build():
    import concourse.mybir as mybir
    import concourse.tile as tile
    from concourse import bacc

    f32 = mybir.dt.float32
    bf16 = mybir.dt.bfloat16

    nc = bacc.Bacc("TRN2", target_bir_lowering=False, debug=False, num_devices=N_CORES)
    xT = nc.dram_tensor("xT", [128, KT, T_SPAN], bf16, kind="ExternalInput").ap()
    Bm = nc.dram_tensor("Bm", [MT, 128, KT, 128], bf16, kind="ExternalInput").ap()
    a = nc.dram_tensor("a", [128, MT], f32, kind="ExternalInput").ap()
    out = nc.dram_tensor("out", [MT, 128, T_CHUNK], bf16, kind="ExternalOutput").ap()

    with tile.TileContext(nc) as tc, ExitStack() as ctx:
        const = ctx.enter_context(tc.tile_pool(name="const", bufs=1))
        xt_pool = ctx.enter_context(tc.tile_pool(name="xt", bufs=NP))
        b_pool = ctx.enter_context(tc.tile_pool(name="bm", bufs=MT))
        ht_pool = ctx.enter_context(tc.tile_pool(name="ht", bufs=6))
        ps_gemm = ctx.enter_context(tc.tile_pool(name="psg", bufs=8, space="PSUM"))

        rings = [nc.sync, nc.scalar]

        bms = {}
        xps = [None] * NP

        def load_xp(p, ring):
            t = xt_pool.tile([128, 2 * T_SPAN], bf16, tag="xt", name=f"xp{p}")
            ring.dma_start(
                out=t[:].rearrange("p (k t) -> p k t", k=2),
                in_=xT[:, 2 * p : 2 * p + 2, :],
            )
            xps[p] = t

        def load_bm(m, ring, lo=0, hi=KT):
            if m not in bms:
                bms[m] = b_pool.tile([128, KT * 128], bf16, tag="bm", name=f"bm{m}")
            ring.dma_start(
                out=bms[m][:, lo * 128 : hi * 128].rearrange(
                    "p (k c) -> p k c", k=hi - lo
                ),
                in_=Bm[m, :, lo:hi, :],
            )

        # Supply plan in PE-consumption order across both rings (A=sync,
        # B=scalar).  Phase-1 needs xp0-3 + the k0-7 halves of bm0-2
        # first; the k8-15 halves and bm3+ stream behind.
        # Supply plan in PE-consumption order across both rings (A=sync,
        # B=scalar).  Phase-1 needs xp0-3 + the k0-7 halves of bm0-2
        # first; the k8-15 halves and bm3+ stream behind.
        a_sb = const.tile([128, MT], f32)
        load_bm(0, nc.sync, 0, 8)       # A: bm0a
        nc.scalar.dma_start(out=a_sb, in_=a)  # B: a (tiny)
        load_xp(0, nc.scalar)           # B: xp0
        load_xp(1, nc.sync)             # A: xp1
        load_xp(2, nc.scalar)           # B: xp2
        load_xp(3, nc.sync)             # A: xp3
        load_bm(2, nc.scalar, 0, 8)     # B: bm2a
        load_bm(1, nc.sync, 0, 8)      # A: bm1a
        load_xp(4, nc.scalar)           # B: xp4
        load_xp(5, nc.sync)             # A: xp5
        load_xp(6, nc.scalar)           # B: xp6
        load_bm(0, nc.sync, 8, 16)      # A: bm0b
        load_bm(1, nc.scalar, 8, 16)    # B: bm1b
        load_xp(7, nc.sync)             # A: xp7
        load_bm(2, nc.sync, 8, 16)      # A: bm2b
        load_bm(3, nc.sync)             # A
        load_bm(4, nc.scalar)           # B
        load_bm(5, nc.sync)             # A
        load_bm(6, nc.scalar)           # B

        def xt_slice(k, lo, hi):
            return xps[k // 2][:, (k % 2) * T_SPAN + lo : (k % 2) * T_SPAN + hi]

        def emit_mm(ps, m, k, lo, hi):
            nc.tensor.matmul(
                ps[:],
                bms[m][:, k * 128 : (k + 1) * 128],
                xt_slice(k, lo, hi),
                start=(k == 0),
                stop=(k == KT - 1),
            )

        def emit_scan_out(m, psA, psB):
            ht = ht_pool.tile([128, T_SPAN], bf16, tag="ht", name=f"ht{m}")
            a_bc = a_sb[:, m : m + 1].broadcast_to([128, HALF])
            nc.vector.tensor_tensor_scan(
                ht[:, 0:HALF], a_bc, psA[:], 0.0,
                mybir.AluOpType.mult, mybir.AluOpType.add,
            )
            if m < MT - 1:
                nc.vector.tensor_tensor_scan(
                    ht[:, HALF:T_SPAN], a_bc, psB[:], ht[:, HALF - 1 : HALF],
                    mybir.AluOpType.mult, mybir.AluOpType.add,
                )
                rings[m % 2].dma_start(
                    out=out[m, :, :], in_=ht[:, W:T_SPAN]
                )
            else:
                # last m-tile: store the first half as soon as its scan is
                # done and split the trailing scan+store so the final
                # dependency chain after the last matmul is short
                rings[m % 2].dma_start(
                    out=out[m, :, 0 : HALF - W], in_=ht[:, W:HALF]
                )
                q3 = HALF + HALF // 2
                a_bc_h = a_sb[:, m : m + 1].broadcast_to([128, HALF // 2])
                nc.vector.tensor_tensor_scan(
                    ht[:, HALF:q3], a_bc_h, psB[:, 0 : HALF // 2],
                    ht[:, HALF - 1 : HALF],
                    mybir.AluOpType.mult, mybir.AluOpType.add,
                )
                rings[m % 2].dma_start(
                    out=out[m, :, HALF - W : q3 - W], in_=ht[:, HALF:q3]
                )
                nc.vector.tensor_tensor_scan(
                    ht[:, q3:T_SPAN], a_bc_h, psB[:, HALF // 2 : HALF],
                    ht[:, q3 - 1 : q3],
                    mybir.AluOpType.mult, mybir.AluOpType.add,
                )
                rings[(m + 1) % 2].dma_start(
                    out=out[m, :, q3 - W : T_CHUNK], in_=ht[:, q3:T_SPAN]
                )

        # PSUM tiles for phase-1 (A/B halves per m; accumulation groups
        # stay open across the k-half blocks).
        ph1 = {}
        for m in range(PH1):
            ph1[m] = (
                ps_gemm.tile([128, HALF], f32, tag="ps", name=f"psA{m}"),
                ps_gemm.tile([128, HALF], f32, tag="ps", name=f"psB{m}"),
            )

        # Up-front HAM warm-up: filler matmuls on a memset bf16 tile with
        # no DMA dependency, targeting psA0 — m0k0's start=True resets it.
        warm = const.tile([128, HALF], bf16)
        nc.gpsimd.memset(warm, 0.0)
        for _ in range(N_WARM):
            nc.tensor.matmul(
                ph1[0][0][:], warm[:, 0:128], warm[:], start=True, stop=True
            )
        # Spare bank for mid-phase bridge fillers (phase-1 banks all hold
        # open accumulations, so bridges need their own target).
        ps_bridge = ps_gemm.tile([128, HALF], f32, tag="ps", name="ps_bridge")

        def bridge(n):
            # Small fillers that keep the PE busy (and the HAM clock gate
            # warm) across a supply-paced stall; drain at ~56ns each if
            # the data is already resident.
            for _ in range(n):
                nc.tensor.matmul(
                    ps_bridge[:, 0:128], warm[:, 0:128], warm[:, 0:128],
                    start=True, stop=True,
                )

        # Phase 1: m0-m2 in k-half blocks following the x pieces.
        for m in range(PH1):
            for k in range(8):
                emit_mm(ph1[m][0], m, k, 0, HALF)
                emit_mm(ph1[m][1], m, k, HALF, T_SPAN)
            bridge(1 if m < 2 else 0)
        for m in range(PH1):
            for k in range(8, KT):
                emit_mm(ph1[m][0], m, k, 0, HALF)
                emit_mm(ph1[m][1], m, k, HALF, T_SPAN)
        for m in range(PH1):
            emit_scan_out(m, *ph1[m])

        # Phase 2: remaining m-tiles run dense, k-inner; B tiles stream
        # three m ahead, alternating rings.
        for m in range(PH1, MT):
            if m + 4 < MT:
                load_bm(m + 4, rings[(m + 4) % 2])
            psA = ps_gemm.tile([128, HALF], f32, tag="ps", name=f"psA{m}")
            psB = ps_gemm.tile([128, HALF], f32, tag="ps", name=f"psB{m}")
            for k in range(KT):
                emit_mm(psA, m, k, 0, HALF)
            for k in range(KT):
                emit_mm(psB, m, k, HALF, T_SPAN)
            emit_scan_out(m, psA, psB)

    nc.compile()
    return nc


def _get_nc():
    if "nc" not in _CACHE:
        _CACHE["nc"] = _build()
    return _CACHE["nc"]


def _shard_inputs(x, a, B):
    import ml_dtypes

    bf16 = ml_dtypes.bfloat16
    x = np.ascontiguousarray(x, dtype=np.float32)
    a = np.ascontiguousarray(a, dtype=np.float32)
    B = np.ascontiguousarray(B, dtype=np.float32)
    B_lin = np.ascontiguousarray(
        B.reshape(KT, 128, MT, 128).transpose(2, 1, 0, 3).astype(bf16)
    )  # [m, p, k, c] = B[128k+p, 128m+c]
    a_lin = np.ascontiguousarray(a.reshape(MT, 128).T)  # [p, m] = a[128m+p]
    xp = np.concatenate([np.zeros((W, H), np.float32), x], axis=0).astype(bf16)
    in_maps = []
    for c in range(N_CORES):
        chunk = xp[c * T_CHUNK : c * T_CHUNK + T_SPAN]  # (T_SPAN, H)
        xT_lin = np.ascontiguousarray(
            chunk.T.reshape(KT, 128, T_SPAN).transpose(1, 0, 2)
        )  # [p, k, t] = x[t, 128k+p]
        in_maps.append({"xT": xT_lin, "Bm": B_lin, "a": a_lin})
    return in_maps


def _gather_output(results):
    out = np.empty((T_FULL, H), np.float32)
    for c in range(N_CORES):
        o = np.asarray(results[c]["out"], dtype=np.float32)  # (MT, 128, T_CHUNK)
        out[c * T_CHUNK : (c + 1) * T_CHUNK] = o.reshape(H, T_CHUNK).T
    return out[None]


def _run(inputs, trace=False):
    from concourse import bass_utils

    nc = _get_nc()
    in_maps = _shard_inputs(inputs["x"], inputs["a"], inputs["B"])
    res = bass_utils.run_bass_kernel_spmd(
        nc, in_maps, core_ids=list(range(N_CORES)), trace=trace
    )
    return _gather_output(res.results), res


def kernel(x, a, B):
    out, _ = _run({"x": x, "a": a, "B": B})
    return out
